# revision 52
# baseline (speedup 1.0000x reference)
"""Trainium2 Bass kernel for RAFT BasicUpdateBlock (dense CNN stack + SepConvGRU).

Strategy:
  - Pure data parallel: batch 8 -> 1 sample per NeuronCore (8 cores).
  - Every conv is shift-and-matmul: out[co, y, x] += W_tap[ci,co].T @ in[ci, y+dy, x+dx]
    accumulated in PSUM over taps x cin-chunks, with edge taps clipped via
    strided APs (zero-padding semantics emerge from the skipped contributions).
  - Activations live in SBUF as [C, H*W] bf16 tiles; weights are packed
    host-side into three [128, ncols] bf16 lhsT atlases (encoder/GRU/head),
    loaded per phase to fit the 192 KB/partition SBUF budget.
  - PSUM eviction fuses bias + nonlinearity on ScalarE (sigmoid/tanh/relu) or
    VectorE (relu/identity via tensor_scalar), with dtype cast on the way out.
  - convf1 (7x7 on 2 channels) is im2col'd on the host into a [98, HW] input.
"""

import numpy as np
import ml_dtypes
from contextlib import ExitStack

import concourse.bass as bass
import concourse.tile as tile
from concourse import bacc, mybir
from concourse.bass_utils import run_bass_kernel_spmd

BF = ml_dtypes.bfloat16
F32 = np.float32
H, W = 64, 128
HW = H * W
TR = 4          # rows per 512-col tile
TW = TR * W     # 512
NT = HW // TW   # 16

AF = mybir.ActivationFunctionType
ALU = mybir.AluOpType

# name -> (kh, kw, cin_chunks [(ci_lo, ci_c)], cout_chunks [(co_lo, co_c)])
SPECS = {
    'convc1': (1, 1, [(0, 128), (128, 128), (256, 68)], [(0, 128), (128, 128)]),
    'convc2': (3, 3, [(0, 128), (128, 128)], [(0, 128), (128, 64)]),
    'convf1': (1, 1, [(0, 98)], [(0, 128)]),      # im2col'd 7x7x2 -> 98
    'convf2': (3, 3, [(0, 128)], [(0, 64)]),
    'conv':   (3, 3, [(0, 128), (128, 128)], [(0, 126)]),
    'convz1': (1, 5, [(0, 128), (128, 128), (256, 128)], [(0, 128)]),
    'convr1': (1, 5, [(0, 128), (128, 128), (256, 128)], [(0, 128)]),
    'convq1': (1, 5, [(0, 128), (128, 128), (256, 128)], [(0, 128)]),
    'convz2': (5, 1, [(0, 128), (128, 128), (256, 128)], [(0, 128)]),
    'convr2': (5, 1, [(0, 128), (128, 128), (256, 128)], [(0, 128)]),
    'convq2': (5, 1, [(0, 128), (128, 128), (256, 128)], [(0, 128)]),
    'fh1':    (3, 3, [(0, 128)], [(0, 128), (128, 128)]),
    # fh2 (3x3, 256->2) decomposed: fh2a = (1,3) conv to 6 packed (dy, c)
    # channels; fh2b = (3,1) selector conv summing the dy-shifted rows.
    'fh2a':   (1, 3, [(0, 128), (128, 128)], [(0, 6)]),
    'fh2b':   (3, 1, [(0, 6)], [(0, 2)]),
    'mask1':  (3, 3, [(0, 128)], [(0, 128), (128, 128)]),
    'mask2':  (1, 1, [(0, 128), (128, 128)],
               [(0, 128), (128, 128), (256, 128), (384, 128), (512, 64)]),
}
WORDER = ['convc1', 'convf1', 'convc2', 'convf2', 'conv',
          'convz1', 'convr1', 'convq1', 'convz2', 'convr2', 'convq2',
          'fh1', 'fh2a', 'fh2b', 'mask1', 'mask2']
PIECE = {n: (0 if i < 5 else (1 if i < 11 else 2))
         for i, n in enumerate(WORDER)}
# psum/output partition base per layer (convf2 lands at partitions 64:128 of
# the shared [cor2_hi | flo2] tile; fh2a/fh2b sit at col-groups 2/3 so they
# run concurrently with mask2's last 64-wide chunk at col-group 0-1)
PBASE = {n: 0 for n in WORDER}
PBASE['convf2'] = 64
# partition rows the lhsT occupies in the weight atlas (must match the rhs
# partition base for K-alignment in the PE array)
WROW = {n: 0 for n in WORDER}


def _taps(kh, kw):
    out = [(0, 0)]
    for dy in range(-(kh // 2), kh // 2 + 1):
        for dx in range(-(kw // 2), kw // 2 + 1):
            if (dy, dx) != (0, 0):
                out.append((dy, dx))
    return out


def pack_weights(params):
    """Returns (atlases [3 arrays [128, ncols] bf16], B_atlas [128, nbias] f32,
    woffs {(name,coi,ti,cii): (piece, col, ci_c, co_c)}, boffs {(name,coi): col})."""
    params = dict(params)
    w2, b2 = params.pop('fh2')
    w2 = np.asarray(w2, F32)
    wa = np.zeros((6, 256, 1, 3), F32)
    wb = np.zeros((2, 6, 3, 1), F32)
    for dy in range(3):
        for c in range(2):
            wa[dy * 2 + c, :, 0, :] = w2[c, :, dy, :]
            wb[c, dy * 2 + c, dy, 0] = 1.0
    params['fh2a'] = (wa, np.zeros(6, F32))
    params['fh2b'] = (wb, np.asarray(b2, F32))

    blocks = [[], [], []]
    cols = [0, 0, 0]
    woffs = {}
    bias_cols, boffs = [], {}
    for name in WORDER:
        kh, kw, cins, couts = SPECS[name]
        pc = PIECE[name]
        w, b = params[name]
        w = np.asarray(w, F32)
        b = np.asarray(b, F32)
        if name == 'convf1':
            # [128, 2, 7, 7] -> [128, 98, 1, 1] rows ordered (ky, kx, c)
            w = w.transpose(0, 2, 3, 1).reshape(128, 98)[:, :, None, None]
        pb = PBASE[name]
        for coi, (co_lo, co_c) in enumerate(couts):
            for ti, (dy, dx) in enumerate(_taps(kh, kw)):
                ky, kx = dy + kh // 2, dx + kw // 2
                for cii, (ci_lo, ci_c) in enumerate(cins):
                    blk = np.zeros((128, co_c), F32)
                    rb = WROW[name]
                    blk[rb:rb + ci_c, :] = w[co_lo:co_lo + co_c,
                                             ci_lo:ci_lo + ci_c, ky, kx].T
                    blocks[pc].append(blk)
                    woffs[(name, coi, ti, cii)] = (pc, cols[pc], ci_c, co_c)
                    cols[pc] += co_c
            bc = np.zeros(128, F32)
            bvals = b[co_lo:co_lo + co_c]
            if name == 'mask2':
                bvals = 0.25 * bvals
            bc[pb:pb + co_c] = bvals
            boffs[(name, coi)] = len(bias_cols)
            bias_cols.append(bc)
    atlases = [np.ascontiguousarray(np.concatenate(bl, 1).astype(BF))
               for bl in blocks]
    B_atlas = np.ascontiguousarray(np.stack(bias_cols, 1))
    return atlases, B_atlas, woffs, boffs


def im2col_flow(flow):
    """flow [2, 64, 128] f32 -> [98, HW] bf16, rows ordered (ky, kx, c)."""
    pad = np.zeros((2, H + 6, W + 6), F32)
    pad[:, 3:3 + H, 3:3 + W] = flow
    rows = np.empty((98, HW), F32)
    for ky in range(7):
        for kx in range(7):
            for c in range(2):
                rows[(ky * 7 + kx) * 2 + c] = pad[c, ky:ky + H, kx:kx + W].reshape(HW)
    return rows.astype(BF)


def _emit(nc, tc, dram, woffs, boffs):
    f32, bf16 = mybir.dt.float32, mybir.dt.bfloat16
    wt = [None, None, None]   # per-phase weight atlas SBUF tiles
    bt = None

    def v3(ap):
        return ap.rearrange("c (h w) -> c h w", w=W)

    def evict(name, coi, ps, dst_cols_ap, func, engine, scale=1.0):
        _, _, _, couts = SPECS[name]
        co_lo, co_c = couts[coi]
        pb = PBASE[name]
        bias_ap = bt[pb:pb + co_c, boffs[(name, coi)]:boffs[(name, coi)] + 1]
        src = ps[pb:pb + co_c, :]
        if engine == 'act':
            nc.scalar.activation(dst_cols_ap, src, func, bias=bias_ap, scale=scale)
        else:  # DVE tensor_scalar: (in + bias) op1 s2
            if func == AF.Relu:
                nc.vector.tensor_scalar(dst_cols_ap, src, bias_ap, 0.0,
                                        ALU.add, ALU.max)
            elif scale == 1.0:
                nc.vector.tensor_scalar_add(dst_cols_ap, src, bias_ap)
            else:
                # (psum * scale) + bias  (bias already pre-scaled host-side)
                nc.vector.tensor_scalar(dst_cols_ap, src, scale, bias_ap,
                                        ALU.mult, ALU.add)

    def conv_tile(name, t, srcs, dsts, func, engines=None, scale=1.0,
                  co_sel=None):
        """Emit all matmuls + evictions for output tile t of layer `name`.
        srcs: per cin chunk: [ci_c, H, W] AP (resident) or [ci_c, TR, W]
        AP (streamed, covering rows of tile t only).
        dsts: per cout chunk: [co_c, HW] AP (partition-presliced).
        """
        kh, kw, cins, couts = SPECS[name]
        taps = _taps(kh, kw)
        pb = PBASE[name]
        for coi, (co_lo, co_c) in enumerate(couts):
            if co_sel is not None and coi not in co_sel:
                continue
            ps = psum_pool.tile([128, TW], f32, tag="ps", name="ps")
            ps3 = v3(ps)
            items = []
            for ti, (dy, dx) in enumerate(taps):
                a = max(TR * t, max(0, -dy))
                b = min(TR * t + TR, H - max(0, dy))
                x0, x1 = max(0, -dx), W - max(0, dx)
                if a >= b or x0 >= x1:
                    continue
                for cii in range(len(cins)):
                    items.append((ti, dy, dx, a, b, x0, x1, cii))
            n = len(items)
            for i, (ti, dy, dx, a, b, x0, x1, cii) in enumerate(items):
                pc, col, ci_c, _ = woffs[(name, coi, ti, cii)]
                src = srcs[cii]
                if src.shape[1] == H:          # full resident [ci_c, H, W]
                    rhs = src[:, a + dy:b + dy, x0 + dx:x1 + dx]
                else:                          # streamed tile rows TR*t..+TR
                    rhs = src[:, a - TR * t:b - TR * t, x0:x1]
                out = ps3[pb:pb + co_c, a - TR * t:b - TR * t, x0:x1]
                nc.tensor.matmul(out, wt[pc][0:ci_c, col:col + co_c], rhs,
                                 start=(i == 0), stop=(i == n - 1))
            eng = engines[coi] if engines else 'act'
            evict(name, coi, ps, dsts[coi][:, t * TW:(t + 1) * TW], func,
                  eng, scale)

    def load_watlas(pool, pc, nchunks=4):
        ncols = dram[f'w{pc}'].shape[1]
        w = pool.tile([128, ncols], bf16, name=f"watlas{pc}")
        step = -(-ncols // nchunks)
        for a in range(0, ncols, step):
            b = min(a + step, ncols)
            nc.sync.dma_start(w[:, a:b], dram[f'w{pc}'].ap()[:, a:b])
        return w

    with ExitStack() as top:
        psum_pool = top.enter_context(
            tc.tile_pool(name="psum", bufs=8, space="PSUM"))
        pool_misc = top.enter_context(tc.tile_pool(name="misc", bufs=1))

        # PE warmup: dummy matmuls on a zeroed tile bridge the initial DMA
        # wait and flip the HAM clock gate to 8/8 before real work arrives.
        warm = pool_misc.tile([128, TW], bf16, name="warm")
        nc.vector.memset(warm[:], 0)
        for _ in range(25):
            wps = psum_pool.tile([128, TW], f32, tag="ps", name="wps")
            nc.tensor.matmul(wps[:, :], warm[:, 0:128], warm[:, :],
                             start=True, stop=True)

        bt = pool_misc.tile([128, dram['biases'].shape[1]], f32, name="bt")

        pool_h2 = top.enter_context(tc.tile_pool(name="h2pool", bufs=1))
        h2 = pool_h2.tile([128, HW], bf16, name="h2")

        # head weight atlas lives in the permanent region so its DMA
        # (emitted mid-encoder) prefetches long before the head phase
        pool_wh = top.enter_context(tc.tile_pool(name="wphead", bufs=1))
        ncols2 = dram['w2'].shape[1]
        wt[2] = pool_wh.tile([128, ncols2], bf16, name="watlas2")

        mid = top.enter_context(ExitStack())
        pool_x = mid.enter_context(tc.tile_pool(name="xpool", bufs=1))
        inp = pool_x.tile([128, HW], bf16, name="inp")
        motion2 = pool_x.tile([128, HW], bf16, name="motion2")

        # ---------------- encoder ----------------
        with ExitStack() as enc:
            # enc_a sits at the stack base so the GRU weight pool (allocated
            # at the same address after this scope closes) only depends on
            # cor1a/b readers (convc2) — its DMA then overlaps convf/conv.
            enc_a = enc.enter_context(tc.tile_pool(name="enca", bufs=1))
            cor1a = enc_a.tile([128, HW], bf16, name="cor1a")
            cor1b = enc_a.tile([128, HW], bf16, name="cor1b")
            enc_b = enc.enter_context(tc.tile_pool(name="encb", bufs=1))
            cor2a = enc_b.tile([128, HW], bf16, name="cor2a")
            cor2b = enc_b.tile([128, HW], bf16, name="cor2b")  # [cor2_hi|flo2]
            flo1 = enc_b.tile([128, HW], bf16, name="flo1")
            stream = enc.enter_context(tc.tile_pool(name="stream", bufs=4))
            wp_enc = enc.enter_context(tc.tile_pool(name="wpenc", bufs=1))
            # only the first atlas chunk (covers convc1) before the corr
            # stream; the rest follows once convc1's DMA burst is underway
            ncols0 = dram['w0'].shape[1]
            wt[0] = wp_enc.tile([128, ncols0], bf16, name="watlas0")
            step0 = -(-ncols0 // 4)
            nc.sync.dma_start(wt[0][:, 0:step0], dram['w0'].ap()[:, 0:step0])
            nc.sync.dma_start(bt[:], dram['biases'].ap()[:])
            nc.sync.dma_start(wt[0][:, step0:2 * step0],
                              dram['w0'].ap()[:, step0:2 * step0])

            # enc1: convc1 (streamed corr) + convf1 (streamed im2col) +
            # convc2 chunk0 interleaved per tile — dense PE stream, DMA
            # demand spread over the whole loop.
            with nc.named_scope("enc1"):
                corr2d = dram['corr'].ap()
                f98d = dram['f98'].ap()
                chunk_rows = [(0, 128), (128, 128), (256, 68)]
                for t in range(NT + 1):
                    if t < NT:
                        stiles = []
                        for ci, (rlo, rc) in enumerate(chunk_rows):
                            st = stream.tile([128, TW], bf16, tag=f"corr{ci}",
                                             name=f"corr{ci}")
                            nc.sync.dma_start(
                                st[0:rc, :],
                                corr2d[rlo:rlo + rc, t * TW:(t + 1) * TW])
                            stiles.append(
                                st[0:rc, :].rearrange("c (r w) -> c r w", w=W))
                        conv_tile('convc1', t, stiles, [cor1a, cor1b],
                                  AF.Relu, engines=['act', 'dve'])
                        f98t = stream.tile([128, TW], bf16, tag="f98",
                                           name="f98t")
                        nc.sync.dma_start(f98t[0:98, :],
                                          f98d[:, t * TW:(t + 1) * TW])
                        conv_tile('convf1', t,
                                  [f98t[0:98, :].rearrange(
                                      "c (r w) -> c r w", w=W)],
                                  [flo1], AF.Relu)
                        if t == 4:
                            for a in range(2 * step0, ncols0, step0):
                                b = min(a + step0, ncols0)
                                nc.sync.dma_start(wt[0][:, a:b],
                                                  dram['w0'].ap()[:, a:b])
                        if t == 6:
                            nc.sync.dma_start(inp[:], dram['inp'].ap()[:])
                            nc.sync.dma_start(motion2[126:128, :],
                                              dram['flow2'].ap()[:])
                            step2 = -(-ncols2 // 2)
                            for a in range(0, ncols2, step2):
                                b = min(a + step2, ncols2)
                                nc.sync.dma_start(wt[2][:, a:b],
                                                  dram['w2'].ap()[:, a:b])
                    if t >= 1:
                        conv_tile('convc2', t - 1, [v3(cor1a), v3(cor1b)],
                                  [cor2a, cor2b[0:64, :]], AF.Relu,
                                  engines=['act', 'dve'], co_sel=[0])

            # enc2: convc2 chunk1 (M=64, cols 0:63) paired with convf2
            # (M=64, cols 64:127) as concurrent column-tiled matmuls.
            with nc.named_scope("enc2"):
                cins_c2 = SPECS['convc2'][2]
                srcs_cb = [v3(cor1a), v3(cor1b)]
                src_f2 = v3(flo1)

                def clipped_items(ncin, t):
                    out = []
                    for ti, (dy, dx) in enumerate(_taps(3, 3)):
                        a = max(TR * t, max(0, -dy))
                        b = min(TR * t + TR, H - max(0, dy))
                        x0, x1 = max(0, -dx), W - max(0, dx)
                        if a >= b:
                            continue
                        for cii in range(ncin):
                            out.append((ti, dy, dx, a, b, x0, x1, cii))
                    return out

                for t in range(NT):
                    ps_cb = psum_pool.tile([128, TW], f32, tag="ps", name="ps")
                    ps_f2 = psum_pool.tile([128, TW], f32, tag="ps", name="ps")
                    cb3, f23 = v3(ps_cb), v3(ps_f2)
                    it_cb = clipped_items(2, t)
                    it_f2 = clipped_items(1, t)

                    def mm_cb(k, item):
                        ti, dy, dx, a, b, x0, x1, cii = item
                        pc, col, ci_c, _ = woffs[('convc2', 1, ti, cii)]
                        nc.tensor.matmul(
                            cb3[0:64, a - TR * t:b - TR * t, x0:x1],
                            wt[pc][0:ci_c, col:col + 64],
                            srcs_cb[cii][:, a + dy:b + dy, x0 + dx:x1 + dx],
                            start=(k == 0), stop=(k == len(it_cb) - 1),
                            tile_position=(0, 0))

                    def mm_f2(j, item):
                        ti, dy, dx, a, b, x0, x1, cii = item
                        pc, col, ci_c, _ = woffs[('convf2', 0, ti, 0)]
                        nc.tensor.matmul(
                            f23[64:128, a - TR * t:b - TR * t, x0:x1],
                            wt[pc][0:ci_c, col:col + 64],
                            src_f2[:, a + dy:b + dy, x0 + dx:x1 + dx],
                            start=(j == 0), stop=(j == len(it_f2) - 1),
                            tile_position=(0, 64))

                    j = 0
                    for k, item in enumerate(it_cb):
                        mm_cb(k, item)
                        if k % 2 == 1 and j < len(it_f2):
                            mm_f2(j, it_f2[j])
                            j += 1
                    while j < len(it_f2):
                        mm_f2(j, it_f2[j])
                        j += 1
                    cs = slice(t * TW, (t + 1) * TW)
                    evict('convc2', 1, ps_cb, cor2b[0:64, cs], AF.Relu, 'dve')
                    evict('convf2', 0, ps_f2, cor2b[64:128, cs], AF.Relu,
                          'act')
            with nc.named_scope("conv"):
                for t in range(NT):
                    conv_tile('conv', t, [v3(cor2a), v3(cor2b)],
                              [motion2[0:126, :]], AF.Relu)

        # ---------------- GRU ----------------
        LAG = 2
        with ExitStack() as gru:
            wp_gru = gru.enter_context(tc.tile_pool(name="wpgru", bufs=1))
            wt[1] = load_watlas(wp_gru, 1)
            pool_gru = gru.enter_context(tc.tile_pool(name="grupool", bufs=1))
            h0 = pool_gru.tile([128, HW], bf16, name="h0")
            nc.sync.dma_start(h0[:], dram['net'].ap()[:])
            h1 = pool_gru.tile([128, HW], bf16, name="h1")

            def gru_pass(idx, h_in, h_out):
                zl, rl, ql = (f'convz{idx}', f'convr{idx}', f'convq{idx}')
                z = pool_gru.tile([128, HW], bf16, tag="z", name="z")
                r = pool_gru.tile([128, HW], bf16, tag="r", name="r")
                q = pool_gru.tile([128, HW], bf16, tag="q", name="q")
                hx_srcs = [v3(h_in), v3(inp), v3(motion2)]
                q_srcs = [v3(r), v3(inp), v3(motion2)]

                def upd(t):
                    cs = slice(t * TW, (t + 1) * TW)
                    nc.vector.tensor_sub(q[:, cs], q[:, cs], h_in[:, cs])
                    nc.vector.tensor_mul(q[:, cs], q[:, cs], z[:, cs])
                    nc.vector.tensor_add(h_out[:, cs], h_in[:, cs], q[:, cs])

                with nc.named_scope(f"gru{idx}"):
                    for t in range(NT + LAG):
                        if t < NT:
                            conv_tile(zl, t, hx_srcs, [z], AF.Sigmoid)
                            conv_tile(rl, t, hx_srcs, [r], AF.Sigmoid)
                            cs = slice(t * TW, (t + 1) * TW)
                            nc.vector.tensor_mul(r[:, cs], r[:, cs],
                                                 h_in[:, cs])
                        if t >= LAG:
                            tq = t - LAG
                            conv_tile(ql, tq, q_srcs, [q], AF.Tanh)
                            upd(tq)

            gru_pass(1, h0, h1)
            gru_pass(2, h1, h2)

        mid.close()   # release inp/motion2 before the head phase

        # ---------------- heads ----------------
        with ExitStack() as head:
            pool_head = head.enter_context(tc.tile_pool(name="headpool",
                                                        bufs=1))
            f1a = pool_head.tile([128, HW], bf16, name="f1a")
            f1b = pool_head.tile([128, HW], bf16, name="f1b")
            m1a = pool_head.tile([128, HW], bf16, name="m1a")
            m1b = pool_head.tile([128, HW], bf16, name="m1b")
            pool_stage = head.enter_context(tc.tile_pool(name="stage", bufs=6))
            pool_out = head.enter_context(tc.tile_pool(name="outp", bufs=1))

            # h2 -> f32 out (DVE cast) while PE runs heads
            QW = HW // 4
            EW = HW // 8
            with nc.named_scope("out_h"):
                for qt in range(4):
                    hs = pool_stage.tile([128, QW], f32, tag="stgh", bufs=2,
                                         name="hs")
                    cs = slice(qt * QW, (qt + 1) * QW)
                    nc.vector.tensor_copy(hs[:], h2[:, cs])
                    nc.sync.dma_start(dram['out_h'].ap()[:, cs], hs[:])

            # fh1/mask1/fh2/mask2 fully interleaved per tile so the 18 MB
            # f32 mask output DMA spreads across the whole head phase.
            # fh2a (M=6, cols 64:70), fh2b (M=2, cols 96:98, lag 2) and
            # mask2 chunk 4 (M=64, cols 0:64) share the PE array via
            # column tiling.
            dflt = pool_out.tile([2, HW], f32, name="dflt")
            A6 = pool_out.tile([6, HW], bf16, name="A6")
            m2_couts = SPECS['mask2'][3]
            m2_cins = SPECS['mask2'][2]
            ms_tiles = [None] * len(m2_couts)
            with nc.named_scope("heads"):
                for t in range(NT + 1):
                    if t < NT:
                        conv_tile('fh1', t, [v3(h2)], [f1a, f1b], AF.Relu,
                                  engines=['act', 'dve'])
                        conv_tile('mask1', t, [v3(h2)], [m1a, m1b], AF.Relu,
                                  engines=['act', 'dve'])
                        conv_tile('fh2a', t, [v3(f1a), v3(f1b)], [A6],
                                  AF.Identity)
                    if t >= 1:
                        conv_tile('fh2b', t - 1, [v3(A6)], [dflt],
                                  AF.Identity)
                    if t < NT:
                        tt, qt = t % 2, t // 2
                        for coi, (co_lo, co_c) in enumerate(m2_couts):
                            if tt == 0:
                                ms_tiles[coi] = pool_stage.tile(
                                    [128, EW], f32, tag="stgm", bufs=6,
                                    name="ms")
                            ps = psum_pool.tile([128, TW], f32, tag="ps",
                                                name="ps")
                            for i in range(len(m2_cins)):
                                pc, col, ci_c, _ = woffs[('mask2', coi, 0, i)]
                                src = [m1a, m1b][i]
                                nc.tensor.matmul(
                                    ps[0:co_c, :],
                                    wt[pc][0:ci_c, col:col + co_c],
                                    src[:, t * TW:(t + 1) * TW],
                                    start=(i == 0),
                                    stop=(i == len(m2_cins) - 1))
                            evict('mask2', coi, ps,
                                  ms_tiles[coi][0:co_c, tt * TW:(tt + 1) * TW],
                                  AF.Identity, 'dve' if coi % 2 else 'act',
                                  scale=0.25)
                            if tt == 1:
                                nc.sync.dma_start(
                                    dram['out_mask'].ap()[
                                        co_lo:co_lo + co_c,
                                        qt * EW:(qt + 1) * EW],
                                    ms_tiles[coi][0:co_c, :])
                nc.sync.dma_start(dram['out_delta'].ap()[:], dflt[:])


def build():
    nc = bacc.Bacc("TRN2", target_bir_lowering=False, debug=False,
                   enable_asserts=True, num_devices=8)
    return nc


def run(inputs, trace=False, tmpdir=None):
    net = np.asarray(inputs['net'], F32)
    inp = np.asarray(inputs['inp'], F32)
    corr = np.asarray(inputs['corr'], F32)
    flow = np.asarray(inputs['flow'], F32)
    params = {k: (np.asarray(w, F32), np.asarray(b, F32))
              for k, (w, b) in inputs['params'].items()}

    atlases, B_atlas, woffs, boffs = pack_weights(params)

    nc = build()
    f32, bf16 = mybir.dt.float32, mybir.dt.bfloat16
    dram = {
        'net': nc.dram_tensor("net", [128, HW], bf16, kind="ExternalInput"),
        'inp': nc.dram_tensor("inp", [128, HW], bf16, kind="ExternalInput"),
        'corr': nc.dram_tensor("corr", [324, HW], bf16, kind="ExternalInput"),
        'f98': nc.dram_tensor("f98", [98, HW], bf16, kind="ExternalInput"),
        'flow2': nc.dram_tensor("flow2", [2, HW], bf16, kind="ExternalInput"),
        'biases': nc.dram_tensor("biases", list(B_atlas.shape), f32,
                                 kind="ExternalInput"),
        'out_h': nc.dram_tensor("out_h", [128, HW], f32, kind="ExternalOutput"),
        'out_mask': nc.dram_tensor("out_mask", [576, HW], f32,
                                   kind="ExternalOutput"),
        'out_delta': nc.dram_tensor("out_delta", [2, HW], f32,
                                    kind="ExternalOutput"),
    }
    for pc in range(3):
        dram[f'w{pc}'] = nc.dram_tensor(f"w{pc}", list(atlases[pc].shape),
                                        bf16, kind="ExternalInput")

    with tile.TileContext(nc) as tc:
        _emit(nc, tc, dram, woffs, boffs)
    nc.compile()

    in_maps = []
    for b in range(8):
        m = {
            'net': net[b].reshape(128, HW).astype(BF),
            'inp': inp[b].reshape(128, HW).astype(BF),
            'corr': corr[b].reshape(324, HW).astype(BF),
            'f98': im2col_flow(flow[b]),
            'flow2': flow[b].reshape(2, HW).astype(BF),
            'biases': B_atlas,
        }
        for pc in range(3):
            m[f'w{pc}'] = atlases[pc]
        in_maps.append(m)

    res = run_bass_kernel_spmd(nc, in_maps, core_ids=list(range(8)),
                               trace=trace, tmpdir=tmpdir)

    hs = np.stack([res.results[b]['out_h'].reshape(128, H, W) for b in range(8)])
    ms = np.stack([res.results[b]['out_mask'].reshape(576, H, W) for b in range(8)])
    ds = np.stack([res.results[b]['out_delta'].reshape(2, H, W) for b in range(8)])
    return (hs, ms, ds), res


def kernel(**inputs):
    outs, _ = run(inputs, trace=False)
    return outs


# revision 60
# speedup vs baseline: 1.0108x; 1.0108x over previous
"""Trainium2 Bass kernel for RAFT BasicUpdateBlock (dense CNN stack + SepConvGRU).

Strategy:
  - Pure data parallel: batch 8 -> 1 sample per NeuronCore (8 cores).
  - Every conv is shift-and-matmul: out[co, y, x] += W_tap[ci,co].T @ in[ci, y+dy, x+dx]
    accumulated in PSUM over taps x cin-chunks, with edge taps clipped via
    strided APs (zero-padding semantics emerge from the skipped contributions).
  - Activations live in SBUF as [C, H*W] bf16 tiles; weights are packed
    host-side into three [128, ncols] bf16 lhsT atlases (encoder/GRU/head),
    loaded per phase to fit the 192 KB/partition SBUF budget.
  - PSUM eviction fuses bias + nonlinearity on ScalarE (sigmoid/tanh/relu) or
    VectorE (relu/identity via tensor_scalar), with dtype cast on the way out.
  - convf1 (7x7 on 2 channels) is im2col'd on the host into a [98, HW] input.
"""

import numpy as np
import ml_dtypes
from contextlib import ExitStack

import concourse.bass as bass
import concourse.tile as tile
from concourse import bacc, mybir
from concourse.bass_utils import run_bass_kernel_spmd

BF = ml_dtypes.bfloat16
F32 = np.float32
H, W = 64, 128
HW = H * W
TR = 4          # rows per 512-col tile
TW = TR * W     # 512
NT = HW // TW   # 16

AF = mybir.ActivationFunctionType
ALU = mybir.AluOpType

# name -> (kh, kw, cin_chunks [(ci_lo, ci_c)], cout_chunks [(co_lo, co_c)])
SPECS = {
    'convc1': (1, 1, [(0, 128), (128, 128), (256, 68)], [(0, 128), (128, 128)]),
    'convc2': (3, 3, [(0, 128), (128, 128)], [(0, 128), (128, 64)]),
    'convf1': (1, 1, [(0, 98)], [(0, 128)]),      # im2col'd 7x7x2 -> 98
    'convf2': (3, 3, [(0, 128)], [(0, 64)]),
    'conv':   (3, 3, [(0, 128), (128, 128)], [(0, 126)]),
    'convz1': (1, 5, [(0, 128), (128, 128), (256, 128)], [(0, 128)]),
    'convr1': (1, 5, [(0, 128), (128, 128), (256, 128)], [(0, 128)]),
    'convq1': (1, 5, [(0, 128), (128, 128), (256, 128)], [(0, 128)]),
    'convz2': (5, 1, [(0, 128), (128, 128), (256, 128)], [(0, 128)]),
    'convr2': (5, 1, [(0, 128), (128, 128), (256, 128)], [(0, 128)]),
    'convq2': (5, 1, [(0, 128), (128, 128), (256, 128)], [(0, 128)]),
    'fh1':    (3, 3, [(0, 128)], [(0, 128), (128, 128)]),
    # fh2 (3x3, 256->2) decomposed: fh2a = (1,3) conv to 6 packed (dy, c)
    # channels; fh2b = (3,1) selector conv summing the dy-shifted rows.
    'fh2a':   (1, 3, [(0, 128), (128, 128)], [(0, 6)]),
    'fh2b':   (3, 1, [(0, 6)], [(0, 2)]),
    'mask1':  (3, 3, [(0, 128)], [(0, 128), (128, 128)]),
    'mask2':  (1, 1, [(0, 128), (128, 128)],
               [(0, 128), (128, 128), (256, 128), (384, 128), (512, 64)]),
}
WORDER = ['convc1', 'convf1', 'convc2', 'convf2', 'conv',
          'convz1', 'convr1', 'convq1', 'convz2', 'convr2', 'convq2',
          'fh1', 'fh2a', 'fh2b', 'mask1', 'mask2']
PIECE = {n: (0 if i < 5 else (1 if i < 11 else 2))
         for i, n in enumerate(WORDER)}
# psum/output partition base per layer (convf2 lands at partitions 64:128 of
# the shared [cor2_hi | flo2] tile; fh2a/fh2b sit at col-groups 2/3 so they
# run concurrently with mask2's last 64-wide chunk at col-group 0-1)
PBASE = {n: 0 for n in WORDER}
PBASE['convf2'] = 64
# partition rows the lhsT occupies in the weight atlas (must match the rhs
# partition base for K-alignment in the PE array)
WROW = {n: 0 for n in WORDER}


def _taps(kh, kw):
    out = [(0, 0)]
    for dy in range(-(kh // 2), kh // 2 + 1):
        for dx in range(-(kw // 2), kw // 2 + 1):
            if (dy, dx) != (0, 0):
                out.append((dy, dx))
    return out


def pack_weights(params):
    """Returns (atlases [3 arrays [128, ncols] bf16], B_atlas [128, nbias] f32,
    woffs {(name,coi,ti,cii): (piece, col, ci_c, co_c)}, boffs {(name,coi): col})."""
    params = dict(params)
    w2, b2 = params.pop('fh2')
    w2 = np.asarray(w2, F32)
    wa = np.zeros((6, 256, 1, 3), F32)
    wb = np.zeros((2, 6, 3, 1), F32)
    for dy in range(3):
        for c in range(2):
            wa[dy * 2 + c, :, 0, :] = w2[c, :, dy, :]
            wb[c, dy * 2 + c, dy, 0] = 1.0
    params['fh2a'] = (wa, np.zeros(6, F32))
    params['fh2b'] = (wb, np.asarray(b2, F32))

    blocks = [[], [], []]
    cols = [0, 0, 0]
    woffs = {}
    bias_cols, boffs = [], {}
    for name in WORDER:
        kh, kw, cins, couts = SPECS[name]
        pc = PIECE[name]
        w, b = params[name]
        w = np.asarray(w, F32)
        b = np.asarray(b, F32)
        if name == 'convf1':
            # [128, 2, 7, 7] -> [128, 98, 1, 1] rows ordered (ky, kx, c)
            w = w.transpose(0, 2, 3, 1).reshape(128, 98)[:, :, None, None]
        pb = PBASE[name]
        for coi, (co_lo, co_c) in enumerate(couts):
            for ti, (dy, dx) in enumerate(_taps(kh, kw)):
                ky, kx = dy + kh // 2, dx + kw // 2
                for cii, (ci_lo, ci_c) in enumerate(cins):
                    blk = np.zeros((128, co_c), F32)
                    rb = WROW[name]
                    blk[rb:rb + ci_c, :] = w[co_lo:co_lo + co_c,
                                             ci_lo:ci_lo + ci_c, ky, kx].T
                    blocks[pc].append(blk)
                    woffs[(name, coi, ti, cii)] = (pc, cols[pc], ci_c, co_c)
                    cols[pc] += co_c
            bc = np.zeros(128, F32)
            bvals = b[co_lo:co_lo + co_c]
            if name == 'mask2':
                bvals = 0.25 * bvals
            bc[pb:pb + co_c] = bvals
            boffs[(name, coi)] = len(bias_cols)
            bias_cols.append(bc)
    atlases = [np.ascontiguousarray(np.concatenate(bl, 1).astype(BF))
               for bl in blocks]
    B_atlas = np.ascontiguousarray(np.stack(bias_cols, 1))
    return atlases, B_atlas, woffs, boffs


def im2col_flow(flow):
    """flow [2, 64, 128] f32 -> [98, HW] bf16, rows ordered (ky, kx, c)."""
    pad = np.zeros((2, H + 6, W + 6), F32)
    pad[:, 3:3 + H, 3:3 + W] = flow
    rows = np.empty((98, HW), F32)
    for ky in range(7):
        for kx in range(7):
            for c in range(2):
                rows[(ky * 7 + kx) * 2 + c] = pad[c, ky:ky + H, kx:kx + W].reshape(HW)
    return rows.astype(BF)


def _emit(nc, tc, dram, woffs, boffs):
    f32, bf16 = mybir.dt.float32, mybir.dt.bfloat16
    wt = [None, None, None]   # per-phase weight atlas SBUF tiles
    bt = None

    def v3(ap):
        return ap.rearrange("c (h w) -> c h w", w=W)

    def evict(name, coi, ps, dst_cols_ap, func, engine, scale=1.0):
        _, _, _, couts = SPECS[name]
        co_lo, co_c = couts[coi]
        pb = PBASE[name]
        bias_ap = bt[pb:pb + co_c, boffs[(name, coi)]:boffs[(name, coi)] + 1]
        src = ps[pb:pb + co_c, :]
        if engine == 'act':
            nc.scalar.activation(dst_cols_ap, src, func, bias=bias_ap, scale=scale)
        else:  # DVE tensor_scalar: (in + bias) op1 s2
            if func == AF.Relu:
                nc.vector.tensor_scalar(dst_cols_ap, src, bias_ap, 0.0,
                                        ALU.add, ALU.max)
            elif scale == 1.0:
                nc.vector.tensor_scalar_add(dst_cols_ap, src, bias_ap)
            else:
                # (psum * scale) + bias  (bias already pre-scaled host-side)
                nc.vector.tensor_scalar(dst_cols_ap, src, scale, bias_ap,
                                        ALU.mult, ALU.add)

    def conv_tile(name, t, srcs, dsts, func, engines=None, scale=1.0,
                  co_sel=None):
        """Emit all matmuls + evictions for output tile t of layer `name`.
        srcs: per cin chunk: [ci_c, H, W] AP (resident) or [ci_c, TR, W]
        AP (streamed, covering rows of tile t only).
        dsts: per cout chunk: [co_c, HW] AP (partition-presliced).
        """
        kh, kw, cins, couts = SPECS[name]
        taps = _taps(kh, kw)
        pb = PBASE[name]
        for coi, (co_lo, co_c) in enumerate(couts):
            if co_sel is not None and coi not in co_sel:
                continue
            ps = psum_pool.tile([128, TW], f32, tag="ps", name="ps")
            ps3 = v3(ps)
            items = []
            for ti, (dy, dx) in enumerate(taps):
                a = max(TR * t, max(0, -dy))
                b = min(TR * t + TR, H - max(0, dy))
                x0, x1 = max(0, -dx), W - max(0, dx)
                if a >= b or x0 >= x1:
                    continue
                for cii in range(len(cins)):
                    items.append((ti, dy, dx, a, b, x0, x1, cii))
            n = len(items)
            for i, (ti, dy, dx, a, b, x0, x1, cii) in enumerate(items):
                pc, col, ci_c, _ = woffs[(name, coi, ti, cii)]
                src = srcs[cii]
                if src.shape[1] == H:          # full resident [ci_c, H, W]
                    rhs = src[:, a + dy:b + dy, x0 + dx:x1 + dx]
                else:                          # streamed tile rows TR*t..+TR
                    rhs = src[:, a - TR * t:b - TR * t, x0:x1]
                out = ps3[pb:pb + co_c, a - TR * t:b - TR * t, x0:x1]
                nc.tensor.matmul(out, wt[pc][0:ci_c, col:col + co_c], rhs,
                                 start=(i == 0), stop=(i == n - 1))
            eng = engines[coi] if engines else 'act'
            evict(name, coi, ps, dsts[coi][:, t * TW:(t + 1) * TW], func,
                  eng, scale)

    def load_watlas(pool, pc, nchunks=4):
        ncols = dram[f'w{pc}'].shape[1]
        w = pool.tile([128, ncols], bf16, name=f"watlas{pc}")
        step = -(-ncols // nchunks)
        for a in range(0, ncols, step):
            b = min(a + step, ncols)
            nc.sync.dma_start(w[:, a:b], dram[f'w{pc}'].ap()[:, a:b])
        return w

    with ExitStack() as top:
        psum_pool = top.enter_context(
            tc.tile_pool(name="psum", bufs=8, space="PSUM"))
        pool_misc = top.enter_context(tc.tile_pool(name="misc", bufs=1))

        # PE warmup: dummy matmuls on a zeroed tile bridge the initial DMA
        # wait and flip the HAM clock gate to 8/8 before real work arrives.
        warm = pool_misc.tile([128, TW], bf16, name="warm")
        nc.vector.memset(warm[:], 0)
        for _ in range(25):
            wps = psum_pool.tile([128, TW], f32, tag="ps", name="wps")
            nc.tensor.matmul(wps[:, :], warm[:, 0:128], warm[:, :],
                             start=True, stop=True)

        bt = pool_misc.tile([128, dram['biases'].shape[1]], f32, name="bt")

        pool_h2 = top.enter_context(tc.tile_pool(name="h2pool", bufs=1))
        h2 = pool_h2.tile([128, HW], bf16, name="h2")

        # head weight atlas lives in the permanent region; its DMA (emitted
        # mid-encoder) prefetches ~500us before the head phase
        pool_wh = top.enter_context(tc.tile_pool(name="wphead", bufs=1))
        ncols2 = dram['w2'].shape[1]
        wt[2] = pool_wh.tile([128, ncols2], bf16, name="watlas2")

        mid = top.enter_context(ExitStack())
        pool_x = mid.enter_context(tc.tile_pool(name="xpool", bufs=1))
        inp = pool_x.tile([128, HW], bf16, name="inp")
        motion2 = pool_x.tile([128, HW], bf16, name="motion2")
        h0 = pool_x.tile([128, HW], bf16, name="h0")

        # ---------------- encoder ----------------
        with ExitStack() as enc:
            # enc_a sits at the stack base so the GRU weight pool (allocated
            # at the same address after this scope closes) only depends on
            # cor1a/b readers (convc2) — its DMA then overlaps convf/conv.
            enc_a = enc.enter_context(tc.tile_pool(name="enca", bufs=1))
            cor1a = enc_a.tile([128, HW], bf16, name="cor1a")
            cor1b = enc_a.tile([128, HW], bf16, name="cor1b")
            enc_b = enc.enter_context(tc.tile_pool(name="encb", bufs=1))
            cor2a = enc_b.tile([128, HW], bf16, name="cor2a")
            cor2b = enc_b.tile([128, HW], bf16, name="cor2b")  # [cor2_hi|flo2]
            flo1 = enc_b.tile([128, HW], bf16, name="flo1")
            stream = enc.enter_context(tc.tile_pool(name="stream", bufs=4))
            wp_enc = enc.enter_context(tc.tile_pool(name="wpenc", bufs=1))
            # only the first atlas chunk (covers convc1) before the corr
            # stream; the rest follows once convc1's DMA burst is underway
            ncols0 = dram['w0'].shape[1]
            wt[0] = wp_enc.tile([128, ncols0], bf16, name="watlas0")
            step0 = -(-ncols0 // 4)
            nc.sync.dma_start(wt[0][:, 0:step0], dram['w0'].ap()[:, 0:step0])
            nc.sync.dma_start(bt[:], dram['biases'].ap()[:])
            nc.sync.dma_start(wt[0][:, step0:2 * step0],
                              dram['w0'].ap()[:, step0:2 * step0])

            # enc1: convc1 (streamed corr) + convf1 (streamed im2col) +
            # convc2 chunk0 interleaved per tile — dense PE stream, DMA
            # demand spread over the whole loop.
            with nc.named_scope("enc1"):
                corr2d = dram['corr'].ap()
                f98d = dram['f98'].ap()
                chunk_rows = [(0, 128), (128, 128), (256, 68)]
                for t in range(NT + 1):
                    if t < NT:
                        stiles = []
                        for ci, (rlo, rc) in enumerate(chunk_rows):
                            st = stream.tile([128, TW], bf16, tag=f"corr{ci}",
                                             name=f"corr{ci}")
                            nc.sync.dma_start(
                                st[0:rc, :],
                                corr2d[rlo:rlo + rc, t * TW:(t + 1) * TW])
                            stiles.append(
                                st[0:rc, :].rearrange("c (r w) -> c r w", w=W))
                        conv_tile('convc1', t, stiles, [cor1a, cor1b],
                                  AF.Relu, engines=['act', 'dve'])
                        f98t = stream.tile([128, TW], bf16, tag="f98",
                                           name="f98t")
                        nc.sync.dma_start(f98t[0:98, :],
                                          f98d[:, t * TW:(t + 1) * TW])
                        conv_tile('convf1', t,
                                  [f98t[0:98, :].rearrange(
                                      "c (r w) -> c r w", w=W)],
                                  [flo1], AF.Relu)
                        if t == 4:
                            for a in range(2 * step0, ncols0, step0):
                                b = min(a + step0, ncols0)
                                nc.sync.dma_start(wt[0][:, a:b],
                                                  dram['w0'].ap()[:, a:b])
                        if t == 6:
                            nc.sync.dma_start(inp[:], dram['inp'].ap()[:])
                            nc.sync.dma_start(motion2[126:128, :],
                                              dram['flow2'].ap()[:])
                            nc.sync.dma_start(h0[:], dram['net'].ap()[:])
                            step2 = -(-ncols2 // 2)
                            for a2 in range(0, ncols2, step2):
                                b2 = min(a2 + step2, ncols2)
                                nc.sync.dma_start(wt[2][:, a2:b2],
                                                  dram['w2'].ap()[:, a2:b2])
                    if t >= 1:
                        conv_tile('convc2', t - 1, [v3(cor1a), v3(cor1b)],
                                  [cor2a, cor2b[0:64, :]], AF.Relu,
                                  engines=['act', 'dve'], co_sel=[0])

            # enc2: convc2 chunk1 (M=64, cols 0:63) paired with convf2
            # (M=64, cols 64:127) as concurrent column-tiled matmuls.
            with nc.named_scope("enc2"):
                cins_c2 = SPECS['convc2'][2]
                srcs_cb = [v3(cor1a), v3(cor1b)]
                src_f2 = v3(flo1)

                def clipped_items(ncin, t):
                    out = []
                    for ti, (dy, dx) in enumerate(_taps(3, 3)):
                        a = max(TR * t, max(0, -dy))
                        b = min(TR * t + TR, H - max(0, dy))
                        x0, x1 = max(0, -dx), W - max(0, dx)
                        if a >= b:
                            continue
                        for cii in range(ncin):
                            out.append((ti, dy, dx, a, b, x0, x1, cii))
                    return out

                for t in range(NT):
                    ps_cb = psum_pool.tile([128, TW], f32, tag="ps", name="ps")
                    ps_f2 = psum_pool.tile([128, TW], f32, tag="ps", name="ps")
                    cb3, f23 = v3(ps_cb), v3(ps_f2)
                    it_cb = clipped_items(2, t)
                    it_f2 = clipped_items(1, t)

                    def mm_cb(k, item):
                        ti, dy, dx, a, b, x0, x1, cii = item
                        pc, col, ci_c, _ = woffs[('convc2', 1, ti, cii)]
                        nc.tensor.matmul(
                            cb3[0:64, a - TR * t:b - TR * t, x0:x1],
                            wt[pc][0:ci_c, col:col + 64],
                            srcs_cb[cii][:, a + dy:b + dy, x0 + dx:x1 + dx],
                            start=(k == 0), stop=(k == len(it_cb) - 1),
                            tile_position=(0, 0))

                    def mm_f2(j, item):
                        ti, dy, dx, a, b, x0, x1, cii = item
                        pc, col, ci_c, _ = woffs[('convf2', 0, ti, 0)]
                        nc.tensor.matmul(
                            f23[64:128, a - TR * t:b - TR * t, x0:x1],
                            wt[pc][0:ci_c, col:col + 64],
                            src_f2[:, a + dy:b + dy, x0 + dx:x1 + dx],
                            start=(j == 0), stop=(j == len(it_f2) - 1),
                            tile_position=(0, 64))

                    j = 0
                    for k, item in enumerate(it_cb):
                        mm_cb(k, item)
                        if k % 2 == 1 and j < len(it_f2):
                            mm_f2(j, it_f2[j])
                            j += 1
                    while j < len(it_f2):
                        mm_f2(j, it_f2[j])
                        j += 1
                    cs = slice(t * TW, (t + 1) * TW)
                    evict('convc2', 1, ps_cb, cor2b[0:64, cs], AF.Relu, 'dve')
                    evict('convf2', 0, ps_f2, cor2b[64:128, cs], AF.Relu,
                          'act')
            with nc.named_scope("conv"):
                for t in range(NT):
                    conv_tile('conv', t, [v3(cor2a), v3(cor2b)],
                              [motion2[0:126, :]], AF.Relu)

        # ---------------- GRU ----------------
        LAG = 2
        with ExitStack() as gru:
            wp_gru = gru.enter_context(tc.tile_pool(name="wpgru", bufs=1))
            wt[1] = load_watlas(wp_gru, 1)
            pool_gru = gru.enter_context(tc.tile_pool(name="grupool", bufs=1))
            h1 = pool_gru.tile([128, HW], bf16, name="h1")

            def gru_pass(idx, h_in, h_out):
                zl, rl, ql = (f'convz{idx}', f'convr{idx}', f'convq{idx}')
                z = pool_gru.tile([128, HW], bf16, tag="z", name="z")
                r = pool_gru.tile([128, HW], bf16, tag="r", name="r")
                q = pool_gru.tile([128, HW], bf16, tag="q", name="q")
                hx_srcs = [v3(h_in), v3(inp), v3(motion2)]
                q_srcs = [v3(r), v3(inp), v3(motion2)]

                def upd(t):
                    cs = slice(t * TW, (t + 1) * TW)
                    nc.vector.tensor_sub(q[:, cs], q[:, cs], h_in[:, cs])
                    nc.vector.tensor_mul(q[:, cs], q[:, cs], z[:, cs])
                    nc.vector.tensor_add(h_out[:, cs], h_in[:, cs], q[:, cs])

                with nc.named_scope(f"gru{idx}"):
                    for t in range(NT + LAG):
                        if t < NT:
                            conv_tile(zl, t, hx_srcs, [z], AF.Sigmoid)
                            conv_tile(rl, t, hx_srcs, [r], AF.Sigmoid)
                            cs = slice(t * TW, (t + 1) * TW)
                            nc.vector.tensor_mul(r[:, cs], r[:, cs],
                                                 h_in[:, cs])
                        if t >= LAG:
                            tq = t - LAG
                            conv_tile(ql, tq, q_srcs, [q], AF.Tanh)
                            upd(tq)

            gru_pass(1, h0, h1)
            gru_pass(2, h1, h2)

        mid.close()   # release inp/motion2/h0 before the head phase

        # ---------------- heads ----------------
        with ExitStack() as head:
            pool_head = head.enter_context(tc.tile_pool(name="headpool",
                                                        bufs=1))
            f1a = pool_head.tile([128, HW], bf16, name="f1a")
            f1b = pool_head.tile([128, HW], bf16, name="f1b")
            m1a = pool_head.tile([128, HW], bf16, name="m1a")
            m1b = pool_head.tile([128, HW], bf16, name="m1b")
            pool_stage = head.enter_context(tc.tile_pool(name="stage", bufs=6))
            pool_out = head.enter_context(tc.tile_pool(name="outp", bufs=1))

            # h2 -> f32 out (DVE cast) while PE runs heads
            QW = HW // 4
            EW = HW // 8
            with nc.named_scope("out_h"):
                for qt in range(4):
                    hs = pool_stage.tile([128, QW], f32, tag="stgh", bufs=2,
                                         name="hs")
                    cs = slice(qt * QW, (qt + 1) * QW)
                    nc.vector.tensor_copy(hs[:], h2[:, cs])
                    nc.sync.dma_start(dram['out_h'].ap()[:, cs], hs[:])

            # fh1/mask1/fh2/mask2 fully interleaved per tile so the 18 MB
            # f32 mask output DMA spreads across the whole head phase.
            # fh2a (M=6, cols 64:70), fh2b (M=2, cols 96:98, lag 2) and
            # mask2 chunk 4 (M=64, cols 0:64) share the PE array via
            # column tiling.
            dflt = pool_out.tile([2, HW], f32, name="dflt")
            A6 = pool_out.tile([6, HW], bf16, name="A6")
            m2_couts = SPECS['mask2'][3]
            m2_cins = SPECS['mask2'][2]
            ms_tiles = [None] * len(m2_couts)
            with nc.named_scope("heads"):
                for t in range(NT + 1):
                    if t < NT:
                        conv_tile('fh1', t, [v3(h2)], [f1a, f1b], AF.Relu,
                                  engines=['act', 'dve'])
                        conv_tile('mask1', t, [v3(h2)], [m1a, m1b], AF.Relu,
                                  engines=['act', 'dve'])
                        conv_tile('fh2a', t, [v3(f1a), v3(f1b)], [A6],
                                  AF.Identity)
                    if t >= 1:
                        conv_tile('fh2b', t - 1, [v3(A6)], [dflt],
                                  AF.Identity)
                    if t < NT:
                        tt, qt = t % 2, t // 2
                        for coi, (co_lo, co_c) in enumerate(m2_couts):
                            if tt == 0:
                                ms_tiles[coi] = pool_stage.tile(
                                    [128, EW], f32, tag="stgm", bufs=6,
                                    name="ms")
                            ps = psum_pool.tile([128, TW], f32, tag="ps",
                                                name="ps")
                            for i in range(len(m2_cins)):
                                pc, col, ci_c, _ = woffs[('mask2', coi, 0, i)]
                                src = [m1a, m1b][i]
                                nc.tensor.matmul(
                                    ps[0:co_c, :],
                                    wt[pc][0:ci_c, col:col + co_c],
                                    src[:, t * TW:(t + 1) * TW],
                                    start=(i == 0),
                                    stop=(i == len(m2_cins) - 1))
                            evict('mask2', coi, ps,
                                  ms_tiles[coi][0:co_c, tt * TW:(tt + 1) * TW],
                                  AF.Identity, 'dve' if coi % 2 else 'act',
                                  scale=0.25)
                            if tt == 1:
                                nc.sync.dma_start(
                                    dram['out_mask'].ap()[
                                        co_lo:co_lo + co_c,
                                        qt * EW:(qt + 1) * EW],
                                    ms_tiles[coi][0:co_c, :])
                nc.sync.dma_start(dram['out_delta'].ap()[:], dflt[:])


def build():
    nc = bacc.Bacc("TRN2", target_bir_lowering=False, debug=False,
                   enable_asserts=True, num_devices=8)
    return nc


def run(inputs, trace=False, tmpdir=None):
    net = np.asarray(inputs['net'], F32)
    inp = np.asarray(inputs['inp'], F32)
    corr = np.asarray(inputs['corr'], F32)
    flow = np.asarray(inputs['flow'], F32)
    params = {k: (np.asarray(w, F32), np.asarray(b, F32))
              for k, (w, b) in inputs['params'].items()}

    atlases, B_atlas, woffs, boffs = pack_weights(params)

    nc = build()
    f32, bf16 = mybir.dt.float32, mybir.dt.bfloat16
    dram = {
        'net': nc.dram_tensor("net", [128, HW], bf16, kind="ExternalInput"),
        'inp': nc.dram_tensor("inp", [128, HW], bf16, kind="ExternalInput"),
        'corr': nc.dram_tensor("corr", [324, HW], bf16, kind="ExternalInput"),
        'f98': nc.dram_tensor("f98", [98, HW], bf16, kind="ExternalInput"),
        'flow2': nc.dram_tensor("flow2", [2, HW], bf16, kind="ExternalInput"),
        'biases': nc.dram_tensor("biases", list(B_atlas.shape), f32,
                                 kind="ExternalInput"),
        'out_h': nc.dram_tensor("out_h", [128, HW], f32, kind="ExternalOutput"),
        'out_mask': nc.dram_tensor("out_mask", [576, HW], f32,
                                   kind="ExternalOutput"),
        'out_delta': nc.dram_tensor("out_delta", [2, HW], f32,
                                    kind="ExternalOutput"),
    }
    for pc in range(3):
        dram[f'w{pc}'] = nc.dram_tensor(f"w{pc}", list(atlases[pc].shape),
                                        bf16, kind="ExternalInput")

    with tile.TileContext(nc) as tc:
        _emit(nc, tc, dram, woffs, boffs)
    nc.compile()

    in_maps = []
    for b in range(8):
        m = {
            'net': net[b].reshape(128, HW).astype(BF),
            'inp': inp[b].reshape(128, HW).astype(BF),
            'corr': corr[b].reshape(324, HW).astype(BF),
            'f98': im2col_flow(flow[b]),
            'flow2': flow[b].reshape(2, HW).astype(BF),
            'biases': B_atlas,
        }
        for pc in range(3):
            m[f'w{pc}'] = atlases[pc]
        in_maps.append(m)

    res = run_bass_kernel_spmd(nc, in_maps, core_ids=list(range(8)),
                               trace=trace, tmpdir=tmpdir)

    hs = np.stack([res.results[b]['out_h'].reshape(128, H, W) for b in range(8)])
    ms = np.stack([res.results[b]['out_mask'].reshape(576, H, W) for b in range(8)])
    ds = np.stack([res.results[b]['out_delta'].reshape(2, H, W) for b in range(8)])
    return (hs, ms, ds), res


def kernel(**inputs):
    outs, _ = run(inputs, trace=False)
    return outs
